# revision 1
# baseline (speedup 1.0000x reference)
"""Trainium2 Bass kernel for CoordinationMemory (scatter_memory).

Computation (per batch row n):
    cur_h = memory[n, veh_idx[n], :]
    x     = concat(veh_repr[n], cust_repr[n], edge_emb[n])        # [3D]
    nh    = tanh(x @ W_in + b_in + cur_h @ W_h + b_h)             # [H]
    out   = memory with out[n, veh_idx[n], :] = nh

Full shapes: N=4096, L_V=64, H=512, D=256. Data-parallel over 8 cores
(512 rows each). Per core the output is a 64 MiB copy of the memory
shard with 512 rows (2 KiB each) overwritten — memory-bound; the bulk
copy runs DRAM->DRAM on the SP HWDGE ring while the gather/GEMM/tanh
pipeline runs on the other queues. The output is split into one DRAM
tensor per 128-row tile so each tile's indirect scatter (whose dynamic
AP Tile tracks as a whole-tensor write) only waits for its own tile's
copy and overlaps the rest; measured ~385 us/core, at the pure-copy
floor (~351 GB/s combined HBM R+W, vs ~358 GB/s per-NC spec).

The bias and the x-GEMM are fused by augmenting x with a ones column
(padded to 896 = 7*128 contraction rows) and W_in with a b_in+b_h row.
"""

import numpy as np

import concourse.bass as bass
import concourse.tile as tile
from concourse import bacc, mybir
from concourse.bass_utils import run_bass_kernel_spmd
from concourse.masks import make_identity

N = 4096
LV = 64
H = 512
D = 256
NCORES = 8
NS = N // NCORES          # rows per core
KX = 896                  # padded x contraction dim: 768 data + 1 ones + pad
KXC = KX // 128           # 7 chunks
HC = H // 128             # 4 chunks
P = 128

F32 = mybir.dt.float32
I32 = mybir.dt.int32


def build_program(
    ns=NS,
    lv=LV,
    h=H,
    kx=KX,
    n_copy_chunks=1,
    repeats=1,
    dual_ring=False,
    copy_mode="bounce",  # "d2d" | "bounce"
    bounce_rows=16,
    bounce_bufs=3,
):
    nt = ns // P
    kxc = kx // P
    hc = h // P
    nc = bacc.Bacc(
        "TRN2",
        target_bir_lowering=False,
        debug=False,
        enable_asserts=False,
        num_devices=NCORES,
    )
    mem = nc.dram_tensor("mem", (ns, lv, h), F32, kind="ExternalInput").ap()
    xt = nc.dram_tensor("xt", (kxc, P, ns), F32, kind="ExternalInput").ap()
    wtop = nc.dram_tensor("wtop", (kxc, P, h), F32, kind="ExternalInput").ap()
    wh = nc.dram_tensor("wh", (hc, P, h), F32, kind="ExternalInput").ap()
    # idx[p, t] = row index (relative to tile t's base) for batch row t*128 + p
    idx = nc.dram_tensor("idx", (P, nt), I32, kind="ExternalInput").ap()
    # One output tensor per 128-row tile: the indirect scatter's conservative
    # whole-tensor dep then only covers that tile's own bulk copy, so
    # scatter_t overlaps copy_{t+1}.
    outs = [
        nc.dram_tensor(f"out{t}", (P, lv, h), F32, kind="ExternalOutput").ap()
        for t in range(nt)
    ]

    mem_flat = mem.rearrange("n l h -> (n l) h")
    out_flats = [o.rearrange("n l h -> (n l) h") for o in outs]

    with tile.TileContext(nc) as tc:
        with (
            tc.tile_pool(name="const", bufs=1) as constp,
            tc.tile_pool(name="work", bufs=2) as workp,
            tc.tile_pool(name="stage", bufs=2) as stagep,
            tc.tile_pool(name="psum", bufs=2, space="PSUM") as psump,
            tc.tile_pool(name="psumtr", bufs=2, space="PSUM") as psumtrp,
            tc.tile_pool(name="bounce", bufs=bounce_bufs) as bouncep,
        ):
            ident = constp.tile([P, P], F32)
            make_identity(nc, ident[:])

            # Constants on the ACT HWDGE ring so they don't queue behind the copy.
            xt_sb = constp.tile([P, kxc * ns], F32)
            for c in range(kxc):
                nc.scalar.dma_start(out=xt_sb[:, bass.ts(c, ns)], in_=xt[c])
            wtop_sb = constp.tile([P, kxc * h], F32)
            for c in range(kxc):
                nc.scalar.dma_start(out=wtop_sb[:, bass.ts(c, h)], in_=wtop[c])
            wh_sb = constp.tile([P, hc * h], F32)
            for c in range(hc):
                nc.scalar.dma_start(out=wh_sb[:, bass.ts(c, h)], in_=wh[c])

            def body():
                idx_all = stagep.tile([P, nt], I32)
                nc.scalar.dma_start(out=idx_all[:], in_=idx[:])

                for t in range(nt):
                    if copy_mode == "tri":
                        # rows 0..15 d2d on the SWDGE ring, rest bounced
                        nc.gpsimd.dma_start(
                            out=outs[t][:16], in_=mem[t * P : t * P + 16]
                        )
                        for c in range(1, P // bounce_rows):
                            r0 = c * bounce_rows
                            bt = bouncep.tile([P, bounce_rows * lv * h // P], F32)
                            nc.sync.dma_start(
                                out=bt[:],
                                in_=mem[t * P + r0 : t * P + r0 + bounce_rows]
                                .rearrange("n l h -> (n l h)")
                                .rearrange("(p f) -> p f", p=P),
                            )
                            nc.scalar.dma_start(
                                out=outs[t][r0 : r0 + bounce_rows]
                                .rearrange("n l h -> (n l h)")
                                .rearrange("(p f) -> p f", p=P),
                                in_=bt[:],
                            )
                    elif copy_mode == "bounce":
                        # SBUF bounce: loads on the SP ring, stores on the
                        # ACT ring, so the read and write streams run on
                        # independent descriptor paths.
                        for c in range(P // bounce_rows):
                            r0 = c * bounce_rows
                            bt = bouncep.tile([P, bounce_rows * lv * h // P], F32)
                            nc.sync.dma_start(
                                out=bt[:],
                                in_=mem[t * P + r0 : t * P + r0 + bounce_rows].rearrange(
                                    "n l h -> (n l h)"
                                ).rearrange("(p f) -> p f", p=P),
                            )
                            nc.scalar.dma_start(
                                out=outs[t][r0 : r0 + bounce_rows].rearrange(
                                    "n l h -> (n l h)"
                                ).rearrange("(p f) -> p f", p=P),
                                in_=bt[:],
                            )
                    else:
                        # DRAM->DRAM on the SP HWDGE ring.
                        rpt = P // n_copy_chunks
                        for c in range(n_copy_chunks):
                            eng = (
                                nc.scalar
                                if dual_ring and (t * n_copy_chunks + c) % 2
                                else nc.sync
                            )
                            eng.dma_start(
                                out=outs[t][c * rpt : (c + 1) * rpt],
                                in_=mem[t * P + c * rpt : t * P + (c + 1) * rpt],
                            )

                    cur_h = workp.tile([P, h], F32)
                    nc.gpsimd.indirect_dma_start(
                        out=cur_h[:],
                        out_offset=None,
                        in_=mem_flat[:],
                        in_offset=bass.IndirectOffsetOnAxis(
                            ap=idx_all[:, t : t + 1], axis=0
                        ),
                        element_offset=t * P * lv * h,
                    )

                    # cur_h [n, h] -> cur_hT [h, n] in 128x128 blocks via PE.
                    cur_ht = workp.tile([P, h], F32)
                    for b in range(hc):
                        ptr = psumtrp.tile([P, P], F32, space="PSUM")
                        nc.tensor.transpose(
                            out=ptr[:],
                            in_=cur_h[:, bass.ts(b, P)],
                            identity=ident[:],
                        )
                        nc.vector.tensor_copy(out=cur_ht[:, bass.ts(b, P)], in_=ptr[:])

                    pmm = psump.tile([P, h], F32, space="PSUM")
                    for c in range(kxc):
                        nc.tensor.matmul(
                            out=pmm[:],
                            lhsT=xt_sb[:, c * ns + t * P : c * ns + (t + 1) * P],
                            rhs=wtop_sb[:, bass.ts(c, h)],
                            start=(c == 0),
                            stop=False,
                        )
                    for b in range(hc):
                        nc.tensor.matmul(
                            out=pmm[:],
                            lhsT=cur_ht[:, bass.ts(b, P)],
                            rhs=wh_sb[:, bass.ts(b, h)],
                            start=False,
                            stop=(b == hc - 1),
                        )

                    nh = stagep.tile([P, h], F32)
                    nc.scalar.activation(
                        out=nh[:],
                        in_=pmm[:],
                        func=mybir.ActivationFunctionType.Tanh,
                    )

                    # Scatter this tile's updated rows into its own output
                    # tensor; only waits for copy_t, overlaps copy_{t+1}.
                    nc.gpsimd.indirect_dma_start(
                        out=out_flats[t][:],
                        out_offset=bass.IndirectOffsetOnAxis(
                            ap=idx_all[:, t : t + 1], axis=0
                        ),
                        in_=nh[:],
                        in_offset=None,
                    )

            if repeats == 1:
                body()
            else:
                with tc.For_i(0, repeats, 1):
                    body()

    nc.compile()
    return nc


_PROGRAM = None


def _get_program():
    global _PROGRAM
    if _PROGRAM is None:
        _PROGRAM = build_program()
    return _PROGRAM


def make_in_maps(memory, veh_idx, veh_repr, cust_repr, edge_emb, W_in, b_in, W_h, b_h):
    memory = np.ascontiguousarray(np.asarray(memory, dtype=np.float32))
    veh_idx = np.asarray(veh_idx).astype(np.int64)
    x_cat = np.concatenate(
        (
            np.asarray(veh_repr, dtype=np.float32)[:, 0, :],
            np.asarray(cust_repr, dtype=np.float32)[:, 0, :],
            np.asarray(edge_emb, dtype=np.float32)[:, 0, 0, :],
            np.ones((N, 1), dtype=np.float32),
        ),
        axis=1,
    )  # [N, 769]

    wtop = np.zeros((KX, H), dtype=np.float32)
    wtop[: 3 * D] = np.asarray(W_in, dtype=np.float32)
    wtop[3 * D] = np.asarray(b_in, dtype=np.float32) + np.asarray(b_h, dtype=np.float32)
    wtop = wtop.reshape(KXC, P, H)
    wh = np.ascontiguousarray(np.asarray(W_h, dtype=np.float32)).reshape(HC, P, H)

    nt = NS // P
    in_maps = []
    for s in range(NCORES):
        lo, hi = s * NS, (s + 1) * NS
        xt = np.zeros((KX, NS), dtype=np.float32)
        xt[: 3 * D + 1] = x_cat[lo:hi].T
        # idx[p, t] = p*LV + veh_idx[t*128+p], relative to tile t's base
        v = veh_idx[lo:hi, 0].reshape(nt, P).T
        idx = np.ascontiguousarray(
            (np.arange(P, dtype=np.int64)[:, None] * LV + v).astype(np.int32)
        )
        in_maps.append(
            {
                "mem": memory[lo:hi],
                "xt": np.ascontiguousarray(xt.reshape(KXC, P, NS)),
                "wtop": wtop,
                "wh": wh,
                "idx": idx,
            }
        )
    return in_maps


def kernel(memory, veh_idx, veh_repr, cust_repr, edge_emb, W_in, b_in, W_h, b_h):
    nc = _get_program()
    in_maps = make_in_maps(
        memory, veh_idx, veh_repr, cust_repr, edge_emb, W_in, b_in, W_h, b_h
    )
    res = run_bass_kernel_spmd(nc, in_maps, core_ids=list(range(NCORES)))
    nt = NS // P
    return np.concatenate(
        [r[f"out{t}"] for r in res.results for t in range(nt)], axis=0
    )



# revision 5
# speedup vs baseline: 3.2890x; 3.2890x over previous
"""Trainium2 Bass kernel for CoordinationMemory (scatter_memory).

Computation (per batch row n):
    cur_h = memory[n, veh_idx[n], :]
    x     = concat(veh_repr[n], cust_repr[n], edge_emb[n])        # [3D]
    nh    = tanh(x @ W_in + b_in + cur_h @ W_h + b_h)             # [H]
    out   = memory with out[n, veh_idx[n], :] = nh

Full shapes: N=4096, L_V=64, H=512, D=256. Data-parallel over 8 cores
(512 rows each).

The output is memory with only 512 of 32768 rows rewritten, so the
dominant cost of a naive kernel is the 64 MiB/core DRAM->DRAM copy of the
untouched rows (~375 us at the ~358 GB/s HBM roofline). This kernel
eliminates the copy: the per-core memory shard is DONATED as the
ExternalOutput buffer (XLA input-output aliasing, the same donation
mechanism run_bass_via_pjrt uses for its zero-initialized outputs), so
the NEFF sees `out` pre-populated with the memory contents and only has
to gather the 512 active rows, run the small GEMMs, and scatter the 512
updated rows back in place (~6 MiB of HBM traffic total).

run_bass_kernel_spmd's axon redirect hardcodes zero-filled donated
outputs, so the dispatch below inlines the same shard_map/_bass_exec_p
path with the memory shard in the donated slot instead.

GEMMs run on the PE in float32r (1 cycle/row at >=256-wide moving dim,
4x faster than plain fp32) with fp32 PSUM accumulation. The biases are
applied with a K=1 ones-vector matmul into the same PSUM accumulation.
"""

import numpy as np
import jax
from jax.sharding import Mesh, PartitionSpec

from jax.experimental.shard_map import shard_map

import concourse.bass as bass
import concourse.tile as tile
from concourse import bacc, mybir
from concourse.bass2jax import (
    _bass_exec_p,
    install_neuronx_cc_hook,
    partition_id_tensor,
)
from concourse.masks import make_identity

N = 4096
LV = 64
H = 512
D = 256
NCORES = 8
NS = N // NCORES          # rows per core
P = 128
NT = NS // P              # 4 row-tiles per core
KC = (3 * D) // P         # 6 contraction chunks for x @ W_in
HC = H // P               # 4 contraction chunks for cur_h @ W_h

F32 = mybir.dt.float32
F32R = mybir.dt.float32r
I32 = mybir.dt.int32

IN_NAMES = ("xt", "wtop", "wh", "bsum", "idx")


def build_program(repeats=1):
    nc = bacc.Bacc(
        "TRN2",
        target_bir_lowering=False,
        debug=False,
        enable_asserts=False,
        num_devices=NCORES,
    )
    xt = nc.dram_tensor("xt", (KC, P, NS), F32R, kind="ExternalInput").ap()
    wtop = nc.dram_tensor("wtop", (KC, P, H), F32R, kind="ExternalInput").ap()
    wh = nc.dram_tensor("wh", (HC, P, H), F32R, kind="ExternalInput").ap()
    bsum = nc.dram_tensor("bsum", (1, H), F32R, kind="ExternalInput").ap()
    # idx[p, t] = (t*128 + p)*LV + veh_idx[t*128 + p]  (absolute row in out_flat)
    idx = nc.dram_tensor("idx", (P, NT), I32, kind="ExternalInput").ap()
    out = nc.dram_tensor("out", (NS, LV, H), F32, kind="ExternalOutput").ap()
    out_flat = out.rearrange("n l h -> (n l) h")

    with tile.TileContext(nc) as tc:
        with (
            tc.tile_pool(name="const", bufs=1) as constp,
            tc.tile_pool(name="wts", bufs=2) as wtsp,
            tc.tile_pool(name="gath", bufs=NT + 1) as gathp,
            tc.tile_pool(name="work", bufs=2) as workp,
            tc.tile_pool(name="stage", bufs=2) as stagep,
            tc.tile_pool(name="psum", bufs=2, space="PSUM") as psump,
            tc.tile_pool(name="psumtr", bufs=2, space="PSUM") as psumtrp,
        ):
            ident = constp.tile([P, P], F32)
            make_identity(nc, ident[:])
            ones_f32 = constp.tile([1, P], F32)
            nc.vector.memset(ones_f32[:], 1.0)
            ones = constp.tile([1, P], F32R)
            nc.vector.tensor_copy(out=ones[:], in_=ones_f32[:])

            def body():
                idx_sb = stagep.tile([P, NT], I32)
                nc.sync.dma_start(out=idx_sb[:], in_=idx[:])

                # Constants: x^T on the SP ring, weights on the ACT ring.
                xt_sb = wtsp.tile([P, KC * NS], F32R)
                for c in range(KC):
                    nc.sync.dma_start(out=xt_sb[:, bass.ts(c, NS)], in_=xt[c])
                wtop_sb = wtsp.tile([P, KC * H], F32R)
                for c in range(KC):
                    nc.scalar.dma_start(out=wtop_sb[:, bass.ts(c, H)], in_=wtop[c])
                wh_sb = wtsp.tile([P, HC * H], F32R)
                for c in range(HC):
                    nc.scalar.dma_start(out=wh_sb[:, bass.ts(c, H)], in_=wh[c])
                bs_sb = wtsp.tile([1, H], F32R)
                nc.sync.dma_start(out=bs_sb[:], in_=bsum[:])

                # All gathers up front: they read out_flat, which every
                # scatter below writes; issuing them first keeps the
                # conservative whole-tensor deps from serializing
                # gather_{t+1} behind scatter_t.
                cur_hs = []
                for t in range(NT):
                    cur_h = gathp.tile([P, H], F32)
                    nc.gpsimd.indirect_dma_start(
                        out=cur_h[:],
                        out_offset=None,
                        in_=out_flat[:],
                        in_offset=bass.IndirectOffsetOnAxis(
                            ap=idx_sb[:, t : t + 1], axis=0
                        ),
                    )
                    cur_hs.append(cur_h)

                for t in range(NT):
                    # cur_h [n, h] -> cur_hT [h, n] in 128x128 blocks via PE.
                    cur_ht = workp.tile([P, H], F32R)
                    for b in range(HC):
                        ptr = psumtrp.tile([P, P], F32, space="PSUM")
                        nc.tensor.transpose(
                            out=ptr[:],
                            in_=cur_hs[t][:, bass.ts(b, P)],
                            identity=ident[:],
                        )
                        nc.vector.tensor_copy(
                            out=cur_ht[:, bass.ts(b, P)], in_=ptr[:]
                        )

                    pmm = psump.tile([P, H], F32, space="PSUM")
                    # bias: ones^T @ (b_in + b_h) broadcasts the bias row
                    nc.tensor.matmul(
                        out=pmm[:],
                        lhsT=ones[:],
                        rhs=bs_sb[:],
                        start=True,
                        stop=False,
                    )
                    for c in range(KC):
                        nc.tensor.matmul(
                            out=pmm[:],
                            lhsT=xt_sb[:, c * NS + t * P : c * NS + (t + 1) * P],
                            rhs=wtop_sb[:, bass.ts(c, H)],
                            start=False,
                            stop=False,
                        )
                    for b in range(HC):
                        nc.tensor.matmul(
                            out=pmm[:],
                            lhsT=cur_ht[:, bass.ts(b, P)],
                            rhs=wh_sb[:, bass.ts(b, H)],
                            start=False,
                            stop=(b == HC - 1),
                        )

                    nh = stagep.tile([P, H], F32)
                    nc.scalar.activation(
                        out=nh[:],
                        in_=pmm[:],
                        func=mybir.ActivationFunctionType.Tanh,
                    )
                    nc.gpsimd.indirect_dma_start(
                        out=out_flat[:],
                        out_offset=bass.IndirectOffsetOnAxis(
                            ap=idx_sb[:, t : t + 1], axis=0
                        ),
                        in_=nh[:],
                        in_offset=None,
                    )

            if repeats == 1:
                body()
            else:
                with tc.For_i(0, repeats, 1):
                    body()

    nc.compile()
    return nc


def build_dispatch(nc):
    """jit'd shard_map callable over 8 cores; arg order IN_NAMES + mem
    (donated as the `out` buffer). Returns fn(xt, wtop, wh, bsum, idx, mem)
    -> (out,) with global (axis-0 concatenated) arrays."""
    install_neuronx_cc_hook()
    pname = nc.partition_id_tensor.name if nc.partition_id_tensor else None
    in_names = list(IN_NAMES) + ["out"] + ([pname] if pname else [])
    out_avals = (jax.core.ShapedArray((NS, LV, H), np.float32),)

    def _body(*args):
        ops = list(args)
        if pname:
            ops.append(partition_id_tensor())
        outs = _bass_exec_p.bind(
            *ops,
            out_avals=out_avals,
            in_names=tuple(in_names),
            out_names=("out",),
            lowering_input_output_aliases=(),
            sim_require_finite=True,
            sim_require_nnan=True,
            nc=nc,
        )
        return tuple(outs)

    devices = jax.devices()[:NCORES]
    assert len(devices) == NCORES, f"need {NCORES} cores, have {len(jax.devices())}"
    mesh = Mesh(np.asarray(devices), ("core",))
    nargs = len(IN_NAMES) + 1
    return jax.jit(
        shard_map(
            _body,
            mesh=mesh,
            in_specs=(PartitionSpec("core"),) * nargs,
            out_specs=(PartitionSpec("core"),),
            check_rep=False,
        ),
        donate_argnums=(nargs - 1,),
        keep_unused=True,
    )


def make_global_inputs(
    memory, veh_idx, veh_repr, cust_repr, edge_emb, W_in, b_in, W_h, b_h
):
    """Host-side prep: global (8*per-core axis 0) arrays in IN_NAMES order + mem."""
    mem = np.ascontiguousarray(np.asarray(memory, dtype=np.float32))
    veh = np.asarray(veh_idx).astype(np.int64).reshape(N)
    x_cat = np.concatenate(
        (
            np.asarray(veh_repr, dtype=np.float32)[:, 0, :],
            np.asarray(cust_repr, dtype=np.float32)[:, 0, :],
            np.asarray(edge_emb, dtype=np.float32)[:, 0, 0, :],
        ),
        axis=1,
    )  # [N, 768]
    # xt[core, c, p, n] = x_cat[core*NS + n, c*128 + p]
    xtg = np.ascontiguousarray(
        x_cat.reshape(NCORES, NS, KC, P).transpose(0, 2, 3, 1)
    ).reshape(NCORES * KC, P, NS)
    wtopg = np.ascontiguousarray(
        np.broadcast_to(
            np.asarray(W_in, dtype=np.float32).reshape(1, KC, P, H),
            (NCORES, KC, P, H),
        )
    ).reshape(NCORES * KC, P, H)
    whg = np.ascontiguousarray(
        np.broadcast_to(
            np.asarray(W_h, dtype=np.float32).reshape(1, HC, P, H),
            (NCORES, HC, P, H),
        )
    ).reshape(NCORES * HC, P, H)
    bsumg = np.ascontiguousarray(
        np.broadcast_to(
            (np.asarray(b_in, dtype=np.float32) + np.asarray(b_h, dtype=np.float32))[
                None, :
            ],
            (NCORES, H),
        )
    )
    # idx[core, p, t] = (t*128 + p)*LV + veh[core*NS + t*128 + p]
    base = (np.arange(NT)[None, :, None] * P + np.arange(P)[None, None, :]) * LV
    idxg = (
        (base + veh.reshape(NCORES, NT, P))
        .transpose(0, 2, 1)
        .reshape(NCORES * P, NT)
        .astype(np.int32)
    )
    return [xtg, wtopg, whg, bsumg, idxg, mem]


_PROGRAM = None
_FN = None


def _get_fn():
    global _PROGRAM, _FN
    if _FN is None:
        _PROGRAM = build_program()
        _FN = build_dispatch(_PROGRAM)
    return _FN


def kernel(memory, veh_idx, veh_repr, cust_repr, edge_emb, W_in, b_in, W_h, b_h):
    fn = _get_fn()
    args = make_global_inputs(
        memory, veh_idx, veh_repr, cust_repr, edge_emb, W_in, b_in, W_h, b_h
    )
    (out,) = fn(*args)
    return np.asarray(out)
